# revision 2
# baseline (speedup 1.0000x reference)
"""Trainium2 Bass kernel for nn_Embedding_61366492725854.

Computes einsum('bsi,ie->bse', inputs, embedding) with
B,S,I,E = 64,4096,128,128 — i.e. a (262144,128)@(128,128) f32 matmul.

Strategy (memory-bound, data-parallel over 8 NeuronCores):
  - Flatten inputs to (B*S, I), shard rows evenly: 32768 rows/core.
  - The kernel is HBM-bandwidth bound (~358 GB/s/core). The 2e-2
    tolerance leaves room for bf16 streaming I/O, which halves HBM
    traffic vs f32: the host hands each core its shard pre-transposed
    to XT[i, r] in bf16 (8 MiB), and the device returns the output
    transposed as OUT[e, r] in bf16 (8 MiB); the host casts back.
  - Device pipeline per core:
      DMA in (XT bf16) -> PE matmul with W stationary (loaded once,
      XT moving at N=512/bank) -> PSUM f32 -> VectorE/ScalarE cast
      copy to bf16 SBUF (alternating) -> DMA out.
    W-stationary streams 1 row/cycle through the PE (~14us warm),
    well under the ~47us DMA floor; copies split across DVE+ACT are
    ~10us each. Critical path is pure DMA.
  - In-DMAs issued from SP (sync), out-DMAs from ACT: two separate
    HWDGE rings so reads and writes overlap.
  - Group schedule ramps up (small first transfers start compute
    early) and down (small tail shortens the final-store drain).
"""

import numpy as np
import ml_dtypes

from concourse import bacc, bass, mybir
from concourse import tile
from concourse import bass_utils

B, S, I, E = 64, 4096, 128, 128
N_CORES = 8
ROWS = B * S                 # 262144
R = ROWS // N_CORES          # 32768 rows per core
CHUNK = 512                  # rows per matmul = one PSUM bank (f32)

# group schedule in 512-row chunks: ramp up, steady, ramp down
GROUPS = [2, 2, 4, 8, 8, 8, 8, 8, 8, 4, 2, 2]
assert sum(GROUPS) * CHUNK == R

F32 = mybir.dt.float32
BF16 = mybir.dt.bfloat16
NP_BF16 = ml_dtypes.bfloat16


def _build_nc():
    nc = bacc.Bacc(
        "TRN2",
        target_bir_lowering=False,
        debug=False,
        enable_asserts=False,
        num_devices=N_CORES,
    )
    xt = nc.dram_tensor("xt", [I, R], BF16, kind="ExternalInput")
    w = nc.dram_tensor("w", [I, E], BF16, kind="ExternalInput")
    out = nc.dram_tensor("out", [E, R], BF16, kind="ExternalOutput")

    with tile.TileContext(nc) as tc:
        with (
            tc.tile_pool(name="consts", bufs=1) as consts,
            tc.tile_pool(name="xin", bufs=6) as xin,
            tc.tile_pool(name="outp", bufs=6) as outp,
            tc.tile_pool(name="ps", bufs=8, space=bass.MemorySpace.PSUM) as pso,
        ):
            w_t = consts.tile([I, E], BF16)
            nc.sync.dma_start(w_t[:], w.ap())

            base = 0
            ci = 0
            for g in GROUPS:
                cols = g * CHUNK
                x_t = xin.tile([128, cols], BF16, tag="x_t")
                nc.sync.dma_start(x_t[:], xt.ap()[:, base:base + cols])
                o_t = outp.tile([128, cols], BF16, tag="o_t")
                for j in range(g):
                    ps = pso.tile([128, CHUNK], F32, tag="ps")
                    nc.tensor.matmul(
                        ps[:], w_t[:], x_t[:, j * CHUNK:(j + 1) * CHUNK],
                        start=True, stop=True,
                    )
                    dst = o_t[:, j * CHUNK:(j + 1) * CHUNK]
                    if ci % 2 == 0:
                        nc.vector.tensor_copy(dst, ps[:])
                    else:
                        nc.scalar.copy(dst, ps[:])
                    ci += 1
                nc.scalar.dma_start(out.ap()[:, base:base + cols], o_t[:])
                base += cols

    nc.compile()
    return nc


_cached_nc = None


def _run(X, W, trace=False, trace_kwargs=None):
    """X: (ROWS, I) f32, W: (I, E) f32 -> (ROWS, E) f32 (+ results obj)."""
    global _cached_nc
    if _cached_nc is None:
        _cached_nc = _build_nc()
    nc = _cached_nc
    w16 = np.ascontiguousarray(W.astype(NP_BF16))
    in_maps = [
        {"xt": X[c * R:(c + 1) * R].T.astype(NP_BF16), "w": w16}
        for c in range(N_CORES)
    ]
    res = bass_utils.run_bass_kernel_spmd(
        nc, in_maps, core_ids=list(range(N_CORES)),
        trace=trace, **(trace_kwargs or {}),
    )
    outs = np.concatenate(
        [res.results[c]["out"].T.astype(np.float32) for c in range(N_CORES)],
        axis=0,
    )
    return outs, res


def kernel(inputs, embedding):
    X = np.ascontiguousarray(np.asarray(inputs, dtype=np.float32)).reshape(ROWS, I)
    W = np.ascontiguousarray(np.asarray(embedding, dtype=np.float32))
    outs, _ = _run(X, W)
    return outs.reshape(B, S, E)
